# revision 1
# baseline (speedup 1.0000x reference)
"""Trainium2 Bass kernel for nn_Attention (dense transformer attention block).

Reference computation (per batch b):
  q = BN(wq @ x) -> (8 heads, 16, 3136);  k likewise;  v = BN(wv @ x) -> (8, 64, 3136)
  attn = softmax(q^T k) over 3136x3136 tokens (no 1/sqrt(d) scaling)
  o = attn @ v^T -> (512, 56, 56);  out = BN(wp @ o) -> (256, 56, 56)

Sharding: 8 cores = 2 batches x 4 query-token chunks of 784. Each core
computes k/v for all 3136 key tokens (cheap, redundant) and attention +
output projection for its own 784 query tokens. Zero collectives; the host
assembles the 8 output shards.

Device algorithm per core (flash-style; bf16 matmuls, f32 PSUM accumulation):
  - BN scales folded into weights host-side; biases applied on the PSUM
    evacuation ops (per-partition scalar add, or a broadcast bias tile for
    v'^T whose ones-column doubles as the softmax-denominator accumulator).
  - Scores S_T[m, n] = k_blk^T q (K=16) run as 4-way concurrent PE
    row-group tiles: per-head k/q live at 32-aligned base partitions with
    +32-shifted replicas so consecutive blocks/heads occupy distinct row
    groups (tile_position packing + LDWEIGHTS pull-ahead).
  - exp on ACT straight from PSUM in 3-block batches (bias -7*ln2 is a
    leftover global scale; it cancels in the softmax divide).
  - o'[65, n] += v'^T_blk @ exp(S_T_blk); row 64 accumulates the
    denominator. The o' matmuls of iteration i execute as PE "filler"
    inside iteration i+1's scores/exp phase (software pipeline), so the
    PE never idles waiting on ACT.
  - softmax divide: denominator broadcast via a DRAM-bounce DMA with a
    step-0 partition AP, reciprocal_approx_accurate on DVE, multiply.
  - out = wp_eff @ o; the first n-chunk's projection is emitted early,
    inside the main loop.
"""

import os
import sys

for _p in ("/opt/trn_rl_repo", "/root/.axon_site/_ro/trn_rl_repo"):
    if os.path.isdir(_p) and _p not in sys.path:
        sys.path.insert(0, _p)

import numpy as np

NUM_HEADS = 8
KEY_DIM = 16
D_HEAD = 64
B = 2
C = 256
HH = 56
WW = 56
N = HH * WW          # 3136 tokens
NCHUNK = N // 4      # 784 query tokens per core
NSUB = NCHUNK // 2   # 392, fits one PSUM bank
NB = (N + 127) // 128            # 25 key-blocks
MB_SIZES = [128] * 24 + [64]
KS = [128, 128]                  # contraction chunks for K=256
GROUPS = [list(range(g * 3, min(g * 3 + 3, NB))) for g in range(9)]

_GRAPH = None


def _build_graph():
    import concourse.bass as bass  # noqa: F401
    import concourse.mybir as mybir
    import concourse.tile as tile
    from concourse import bacc
    from contextlib import ExitStack

    f32 = mybir.dt.float32
    bf16 = mybir.dt.bfloat16
    LN2_4 = -4.852030263919617  # exp bias -7*ln2; cancels in the softmax divide
    Exp = mybir.ActivationFunctionType.Exp

    nc = bacc.Bacc("TRN2", target_bir_lowering=False, debug=False, num_devices=8)
    xa_d = nc.dram_tensor("xa", [256, N], bf16, kind="ExternalInput").ap()
    xq_d = nc.dram_tensor("xq", [256, NCHUNK], bf16, kind="ExternalInput").ap()
    wq_d = nc.dram_tensor("wq", [256, 128], bf16, kind="ExternalInput").ap()
    wk_d = nc.dram_tensor("wk", [256, 128], bf16, kind="ExternalInput").ap()
    wv_d = nc.dram_tensor("wv", [256, 520], bf16, kind="ExternalInput").ap()
    qb_d = nc.dram_tensor("qb", [128, 1], f32, kind="ExternalInput").ap()
    kb_d = nc.dram_tensor("kb", [128, 1], f32, kind="ExternalInput").ap()
    vb_d = nc.dram_tensor("vb", [1, 520], bf16, kind="ExternalInput").ap()
    pb_d = nc.dram_tensor("pb", [128, 2], f32, kind="ExternalInput").ap()
    wp_d = nc.dram_tensor("wp", [64, 8, 256], bf16, kind="ExternalInput").ap()
    out_d = nc.dram_tensor("out", [256, NCHUNK], f32, kind="ExternalOutput").ap()
    rsd_d = nc.dram_tensor("rsd", [16, NSUB], f32).ap()  # rowsum bounce

    with tile.TileContext(nc) as tc, ExitStack() as stk:
        const = stk.enter_context(tc.tile_pool(name="const", bufs=1))
        xq_sb = const.tile([128, 2, NCHUNK], bf16, tag="xq")
        wq_sb = const.tile([128, 2, 128], bf16, tag="wq")
        wk_sb = const.tile([128, 2, 128], bf16, tag="wk")
        wv_sb = const.tile([128, 2, 520], bf16, tag="wv")
        wp_sb = const.tile([64, 8, 256], bf16, tag="wp")
        qb_sb = const.tile([128, 1], f32, tag="qb")
        kb_sb = const.tile([128, 1], f32, tag="kb")
        vb_sb = const.tile([128, 2, 260], bf16, tag="vb")
        pb_sb = const.tile([128, 2], f32, tag="pb")
        eb_sb = const.tile([128, 1], f32, tag="eb")
        # per-head 32-aligned base partitions: head h -> (k_lo if h<4 else
        # k_hi) partitions [32*(h%4), 32*(h%4)+16)
        k_lo = const.tile([128, N], bf16, tag="klo")
        k_hi = const.tile([128, N], bf16, tag="khi")
        q_lo = const.tile([128, NCHUNK], bf16, tag="qlo")
        q_hi = const.tile([128, NCHUNK], bf16, tag="qhi")
        # replicas shifted by +32 partitions so consecutive blocks of one head
        # use different PE row groups (4-way concurrent scores)
        k_lo2 = const.tile([128, N], bf16, tag="klo2")
        k_hi2 = const.tile([128, N], bf16, tag="khi2")
        q_lo2 = const.tile([128, NCHUNK], bf16, tag="qlo2")
        q_hi2 = const.tile([128, NCHUNK], bf16, tag="qhi2")
        # v'^T: [m-in-block, block, head-half, 65*hh + (64 v cols + ones col)]
        vT_sb = const.tile([128, NB, 2, 264], bf16, tag="vt")
        of_sb = const.tile([64, 8, NCHUNK], bf16, tag="of")
        y_sb = const.tile([128, 2, NCHUNK], f32, tag="y")

        for kc in range(2):
            ks, off = KS[kc], 128 * kc
            nc.sync.dma_start(out=wq_sb[0:ks, kc, :], in_=wq_d[off:off + ks, :])
            nc.sync.dma_start(out=wk_sb[0:ks, kc, :], in_=wk_d[off:off + ks, :])
            nc.sync.dma_start(out=wv_sb[0:ks, kc, :], in_=wv_d[off:off + ks, :])
            nc.sync.dma_start(out=xq_sb[0:ks, kc, :], in_=xq_d[off:off + ks, :])
        nc.sync.dma_start(out=wp_sb[:], in_=wp_d[:])
        nc.sync.dma_start(out=qb_sb[:], in_=qb_d)
        nc.sync.dma_start(out=kb_sb[:], in_=kb_d)
        nc.sync.dma_start(out=pb_sb[:], in_=pb_d)
        nc.vector.memset(eb_sb[:], LN2_4)
        nc.gpsimd.dma_start(out=vb_sb[:, :, :],
                            in_=vb_d.partition_broadcast(128))

        xa_sb = const.tile([128, 2, N], bf16, tag="xa")
        for kc in range(2):
            nc.sync.dma_start(out=xa_sb[:, kc, :],
                              in_=xa_d[128 * kc:128 * kc + 128, :])

        with tc.tile_pool(name="psA", bufs=2, space="PSUM") as psA, \
             tc.tile_pool(name="psAV", bufs=3, space="PSUM") as psAV, \
             tc.tile_pool(name="sAtmp", bufs=3) as sAtmp, \
             tc.tile_pool(name="tmpA", bufs=1) as tA:
            k_sb = tA.tile([128, N], bf16, tag="ksb")
            q_sb = tA.tile([128, NCHUNK], bf16, tag="qsb")
            for t in (k_lo, k_hi, k_lo2, k_hi2):
                nc.gpsimd.memset(t[:], 0.0)
            for t in (q_lo, q_hi, q_lo2, q_hi2):
                nc.gpsimd.memset(t[:], 0.0)
            # q projection + immediate regroup
            for c2 in range(2):
                q_ps = psA.tile([128, 512], f32, tag="qkps")
                for kc in range(2):
                    nc.tensor.matmul(
                        q_ps[0:128, 0:NSUB],
                        wq_sb[0:KS[kc], kc, :],
                        xq_sb[0:KS[kc], kc, c2 * NSUB:(c2 + 1) * NSUB],
                        start=(kc == 0), stop=(kc == 1))
                nc.scalar.add(
                    q_sb[:, c2 * NSUB:(c2 + 1) * NSUB], q_ps[0:128, 0:NSUB],
                    qb_sb[:, 0:1])
            for h in range(8):
                qt = q_lo if h < 4 else q_hi
                qt2 = q_lo2 if h < 4 else q_hi2
                bp_ = 32 * (h % 4)
                bp2 = (bp_ + 32) % 128
                nc.gpsimd.dma_start(out=qt[bp_:bp_ + 16, :], in_=q_sb[16 * h:16 * h + 16, :])
                nc.gpsimd.dma_start(out=qt2[bp2:bp2 + 16, :], in_=q_sb[16 * h:16 * h + 16, :])
            # k projection, regrouped per 512-column pass
            for p in range(7):
                c0 = 512 * p
                cw = min(512, N - c0)
                k_ps = psA.tile([128, 512], f32, tag="qkps")
                for kc in range(2):
                    nc.tensor.matmul(
                        k_ps[0:128, 0:cw],
                        wk_sb[0:KS[kc], kc, :],
                        xa_sb[0:KS[kc], kc, c0:c0 + cw],
                        start=(kc == 0), stop=(kc == 1))
                nc.scalar.add(k_sb[:, c0:c0 + cw], k_ps[0:128, 0:cw],
                              kb_sb[:, 0:1])
            for h in range(8):
                kt = k_lo if h < 4 else k_hi
                kt2 = k_lo2 if h < 4 else k_hi2
                bp_ = 32 * (h % 4)
                bp2 = (bp_ + 32) % 128
                nc.gpsimd.dma_start(out=kt[bp_:bp_ + 16, :],
                                    in_=k_sb[16 * h:16 * h + 16, :])
                nc.gpsimd.dma_start(out=kt2[bp2:bp2 + 16, :],
                                    in_=k_sb[16 * h:16 * h + 16, :])
            # v'^T projection (runs last; main-loop scores banks WAR on these)
            for mb in range(NB):
                pb = MB_SIZES[mb]
                vt_ps = psAV.tile([128, 2, 512], f32, tag="vtps")
                for half in range(2):
                    for kc in range(2):
                        nc.tensor.matmul(
                            vt_ps[0:pb, half, 0:260],
                            xa_sb[0:KS[kc], kc, mb * 128:mb * 128 + pb],
                            wv_sb[0:KS[kc], kc, half * 260:(half + 1) * 260],
                            start=(kc == 0), stop=(kc == 1))
                if mb % 2 == 0:
                    nc.vector.tensor_add(
                        out=vT_sb[0:pb, mb, :, 0:260],
                        in0=vt_ps[0:pb, :, 0:260], in1=vb_sb[0:pb, :, :])
                else:
                    vtc = sAtmp.tile([128, 2, 260], bf16, tag="vtc")
                    nc.scalar.copy(vtc[0:pb, :, :], vt_ps[0:pb, :, 0:260])
                    nc.vector.tensor_add(
                        out=vT_sb[0:pb, mb, :, 0:260],
                        in0=vtc[0:pb, :, :], in1=vb_sb[0:pb, :, :])

        # main attention loop, software-pipelined:
        # iteration i = (head-pair, n-chunk). During iteration i's scores+exp
        # phase, the PE executes iteration i-1's o'-accumulation matmuls as
        # filler, so it never idles waiting on ACT (keeps HAM warm).
        # PSUM: scores 2 slots x 3 banks + o' 2 slots x 1 bank = 8 banks.
        PAIRS = [(0, 2), (1, 3), (4, 6), (5, 7)]
        ITERS = [(pair, c2) for c2 in range(2) for pair in PAIRS]

        def emit_scores_group(pair, c2, blocks, s_ps2, kts, qts, kts2, qts2,
                              bps, bps2):
            nc0 = c2 * NSUB
            for i, mb in enumerate(blocks):
                pbi = MB_SIZES[mb]
                for e in range(2):
                    if mb % 2 == 0:
                        kte, qte, be = kts[e], qts[e], bps[e]
                    else:
                        kte, qte, be = kts2[e], qts2[e], bps2[e]
                    nc.tensor.matmul(
                        s_ps2[e][0:pbi, i, 0:NSUB],
                        kte[be:be + 32, mb * 128:mb * 128 + pbi],
                        qte[be:be + 32, nc0:nc0 + NSUB],
                        start=True, stop=True,
                        tile_position=(be, 0))

        def make_o_filler(pair, c2, e, p_tile, i, mb, o_ps2):
            def emit():
                h = pair[e]
                c0h = 65 * (h % 4)
                pbi = MB_SIZES[mb]
                nc.tensor.matmul(
                    o_ps2[e][0:65, 0:NSUB],
                    vT_sb[0:pbi, mb, h // 4, c0h:c0h + 65],
                    p_tile[0:pbi, i, 0:NSUB],
                    start=(mb == 0), stop=(mb == NB - 1))
            return emit


        def emit_wp(c2, pool, mos=(0, 1)):
            # contract heads in pipeline-completion order: the last pair's
            # heads (5, 7) come last so earlier matmuls run while the final
            # epilogue's divide chain is still in flight
            nc0 = c2 * NSUB
            KC_ORDER = (0, 2, 1, 3, 4, 6, 5, 7)
            for mo in mos:
                y_ps = pool.tile([128, 512], f32, tag="ops",
                                 name=f"yps{c2}{mo}")
                for j, kc in enumerate(KC_ORDER):
                    nc.tensor.matmul(
                        y_ps[0:128, 0:NSUB],
                        wp_sb[0:64, kc, mo * 128:(mo + 1) * 128],
                        of_sb[0:64, kc, nc0:nc0 + NSUB],
                        start=(j == 0), stop=(j == 7))
                nc.vector.tensor_scalar_add(
                    y_sb[:, mo, nc0:nc0 + NSUB], y_ps[0:128, 0:NSUB],
                    pb_sb[:, mo:mo + 1])

        def emit_epilogue(pair, c2, o_ps2):
            # evacuate PSUM immediately (frees the o' bank for the next
            # iteration's fillers), then run the divide chain from SBUF
            nc0 = c2 * NSUB
            for e in range(2):
                h = pair[e]
                o_ps = o_ps2[e]
                idx = h * 2 + c2
                rsh = pEp.tile([128, NSUB], f32, tag="rsh")
                nc.vector.tensor_copy(rsh[64:65, :], o_ps[64:65, 0:NSUB])
                ou = pEp.tile([64, NSUB], f32, tag="ou")
                nc.scalar.copy(ou[0:64, :], o_ps[0:64, 0:NSUB])
                nc.sync.dma_start(out=rsd_d[idx:idx + 1, :], in_=rsh[64:65, :])
                rb = pEp.tile([64, NSUB], f32, tag="rb")
                nc.sync.dma_start(
                    out=rb[0:64, :],
                    in_=rsd_d[idx:idx + 1, :].partition_broadcast(64))
                rbr = pEp.tile([64, NSUB], f32, tag="rbr")
                scr = pEp.tile([64, NSUB], f32, tag="scr")
                nc.vector.reciprocal_approx_accurate(
                    out=rbr[:], in_=rb[0:64, :], scratch=scr[:])
                nc.vector.tensor_mul(
                    out=of_sb[0:64, h, nc0:nc0 + NSUB],
                    in0=ou[0:64, :], in1=rbr[:])

        with tc.tile_pool(name="pP", bufs=22) as pP, \
             tc.tile_pool(name="pEp", bufs=4) as pEp, \
             tc.tile_pool(name="psO", bufs=2, space="PSUM") as psO, \
             tc.tile_pool(name="psS", bufs=2, space="PSUM") as psS:
            prev = None  # (pair, c2, p_tiles) of the previous iteration
            for it in range(len(ITERS) + 1):
                cur = ITERS[it] if it < len(ITERS) else None
                fillers = []
                if prev is not None:
                    ppair, pc2, p_tiles = prev
                    o_ps2 = [psO.tile([128, 512], f32, tag="ops",
                                      name=f"ops{e}") for e in range(2)]
                    for g2, blocks2 in enumerate(GROUPS):
                        for i2, mb2 in enumerate(blocks2):
                            for e in range(2):
                                fillers.append(make_o_filler(
                                    ppair, pc2, e, p_tiles[g2][e],
                                    i2, mb2, o_ps2))
                if cur is None:
                    for job in fillers:
                        job()
                    emit_epilogue(ppair, pc2, o_ps2)
                    break
                pair, c2 = cur
                kts = [k_lo if h < 4 else k_hi for h in pair]
                qts = [q_lo if h < 4 else q_hi for h in pair]
                kts2 = [k_lo2 if h < 4 else k_hi2 for h in pair]
                qts2 = [q_lo2 if h < 4 else q_hi2 for h in pair]
                bps = [32 * (h % 4) for h in pair]
                bps2 = [(32 * (h % 4) + 32) % 128 for h in pair]
                p_tiles = []
                nfill = len(fillers)
                for g, blocks in enumerate(GROUPS):
                    gsz = len(blocks)
                    pb = MB_SIZES[blocks[-1]]
                    s_ps2 = [psS.tile([128, 3, 512], f32, tag="sps",
                                      name=f"sps{e}") for e in range(2)]
                    emit_scores_group(pair, c2, blocks, s_ps2, kts, qts,
                                      kts2, qts2, bps, bps2)
                    p_sb2 = [pP.tile([128, 3, NSUB], bf16, tag="psb",
                                     name=f"psb{e}") for e in range(2)]
                    for e in range(2):
                        nc.scalar.activation(
                            out=p_sb2[e][0:pb, 0:gsz, 0:NSUB],
                            in_=s_ps2[e][0:pb, 0:gsz, 0:NSUB], func=Exp,
                            bias=eb_sb[0:pb, 0:1])
                    p_tiles.append(p_sb2)
                    # interleave previous iteration's o' matmuls as PE filler
                    ng = len(GROUPS) - 1
                    lo = nfill * max(0, g - 1) // ng
                    hi = nfill * g // ng
                    for job in fillers[lo:hi]:
                        job()
                if prev is not None:
                    emit_epilogue(ppair, pc2, o_ps2)
                    if (ppair, pc2) == (PAIRS[-1], 0):
                        emit_wp(0, psO)
                prev = (pair, c2, p_tiles)

        # output projection for the second n-chunk + store
        with tc.tile_pool(name="psY", bufs=2, space="PSUM") as psY:
            nc0 = NSUB
            for mo in range(2):
                y_ps = psY.tile([128, 512], f32, tag="ops", name=f"yps1{mo}")
                for kc in range(8):
                    nc.tensor.matmul(
                        y_ps[0:128, 0:NSUB],
                        wp_sb[0:64, kc, mo * 128:(mo + 1) * 128],
                        of_sb[0:64, kc, nc0:nc0 + NSUB],
                        start=(kc == 0), stop=(kc == 7))
                nc.vector.tensor_scalar_add(
                    y_sb[:, mo, nc0:nc0 + NSUB], y_ps[0:128, 0:NSUB],
                    pb_sb[:, mo:mo + 1])
                nc.sync.dma_start(
                    out=out_d[mo * 128:(mo + 1) * 128, :], in_=y_sb[:, mo, :])

    nc.compile()
    return nc


def get_graph():
    global _GRAPH
    if _GRAPH is None:
        _GRAPH = _build_graph()
    return _GRAPH


def make_in_maps(x, wq, sq, bq, wk, sk, bk, wv, sv, bv, wp, sp, bp):
    import ml_dtypes
    bf = ml_dtypes.bfloat16
    f = np.float32
    x2 = np.asarray(x, f).reshape(B, C, N)
    ones_row = np.ones((1, N), f)
    wq = np.asarray(wq, f); sq = np.asarray(sq, f); bq = np.asarray(bq, f)
    wk = np.asarray(wk, f); sk = np.asarray(sk, f); bk = np.asarray(bk, f)
    wv = np.asarray(wv, f); sv = np.asarray(sv, f); bv = np.asarray(bv, f)
    wp = np.asarray(wp, f); sp = np.asarray(sp, f); bp = np.asarray(bp, f)

    wq_eff = (wq * sq[:, None]).T.astype(f)           # (256, 128)
    wk_eff = (wk * sk[:, None]).T.astype(f)
    wv_base = wv * sv[:, None]  # (512, 256)
    wv_arr = np.zeros((256, 520), f)
    vb_arr = np.zeros((1, 520), f)
    for h in range(NUM_HEADS):
        col = 260 * (h // 4) + 65 * (h % 4)
        wv_arr[:, col:col + 64] = wv_base[64 * h:64 * h + 64, :].T
        vb_arr[0, col:col + 64] = bv[64 * h:64 * h + 64]
        vb_arr[0, col + 64] = 1.0
    wp_eff = (wp * sp[:, None]).T.astype(f)  # (512, 256), row c = 64h+d
    wp_arr = wp_eff.reshape(8, 64, 256).transpose(1, 0, 2).copy()
    pb_arr = bp.reshape(2, 128).T.copy()  # (128, 2): pb_arr[d, mo] = bp[128*mo+d]
    in_maps = []
    for core in range(8):
        b, j = core // 4, core % 4
        xa_full = np.ascontiguousarray(x2[b])
        xq_c = np.ascontiguousarray(xa_full[:, j * NCHUNK:(j + 1) * NCHUNK])
        in_maps.append(dict(
            xa=xa_full.astype(bf), xq=xq_c.astype(bf),
            wq=wq_eff.astype(bf), wk=wk_eff.astype(bf),
            wv=wv_arr.astype(bf), wp=wp_arr.astype(bf),
            qb=bq.reshape(128, 1).astype(f), kb=bk.reshape(128, 1).astype(f),
            vb=vb_arr.astype(bf), pb=pb_arr.astype(f)))
    return in_maps


def assemble_output(results):
    y = np.zeros((B, C, N), np.float32)
    for core in range(8):
        b, j = core // 4, core % 4
        y[b, :, j * NCHUNK:(j + 1) * NCHUNK] = results[core]["out"]
    return y.reshape(B, C, HH, WW)


def kernel(**inputs):
    from concourse.bass_utils import run_bass_kernel_spmd
    nc = get_graph()
    in_maps = make_in_maps(**inputs)
    res = run_bass_kernel_spmd(nc, in_maps, core_ids=list(range(8)))
    return assemble_output(res.results)


if __name__ == "__main__":
    rng = np.random.default_rng(0)
    ins = dict(
        x=rng.standard_normal((2, 256, 56, 56), np.float32),
        wq=rng.standard_normal((128, 256), np.float32) * 0.05,
        sq=rng.random(128, np.float32),
        bq=rng.standard_normal(128, np.float32) * 0.05,
        wk=rng.standard_normal((128, 256), np.float32) * 0.05,
        sk=rng.random(128, np.float32),
        bk=rng.standard_normal(128, np.float32) * 0.05,
        wv=rng.standard_normal((512, 256), np.float32) * 0.05,
        sv=rng.random(512, np.float32),
        bv=rng.standard_normal(512, np.float32) * 0.05,
        wp=rng.standard_normal((256, 512), np.float32) * 0.05,
        sp=rng.random(256, np.float32),
        bp=rng.standard_normal(256, np.float32) * 0.05,
    )
    out = kernel(**ins)
    print("out", out.shape, out.dtype, float(np.abs(out).mean()))



# revision 7
# speedup vs baseline: 1.1350x; 1.1350x over previous
"""Trainium2 Bass kernel for nn_Attention (dense transformer attention block).

Reference computation (per batch b):
  q = BN(wq @ x) -> (8 heads, 16, 3136);  k likewise;  v = BN(wv @ x) -> (8, 64, 3136)
  attn = softmax(q^T k) over 3136x3136 tokens (no 1/sqrt(d) scaling)
  o = attn @ v^T -> (512, 56, 56);  out = BN(wp @ o) -> (256, 56, 56)

Sharding: 8 cores = 2 batches x 4 query-token chunks of 784. Each core
computes k/v for all 3136 key tokens (cheap, redundant) and attention +
output projection for its own 784 query tokens. Zero collectives; the host
assembles the 8 output shards.

Device algorithm per core (flash-style; bf16 matmuls, f32 PSUM accumulation):
  - The softmax exp (the serial bottleneck: 19.7M elements/core, ACT-only
    at 1 elem/cycle/lane) is split across TWO engines: even score-groups run
    exact exp on ACT; odd groups run a one-op Schraudolph bit-trick on DVE
    (int16(a*S + b) bit-cast to bf16 ~= exp(S)*2^-7). The a=128/ln2 scale is
    folded into wq/bq host-side; ACT's activation-scale undoes it for the
    exact path. The 2^-7 factor cancels in the softmax divide.
  - k-bias is dropped entirely (a per-query-row score shift, softmax-
    invariant); v-bias is folded into the output projection bias host-side
    (out += wp_eff @ bv), so v' evacuation is a plain copy.
  - Scores S_T[m, n] = k_blk^T q (K=16) run as 4-way concurrent PE
    row-group tiles (tile_position packing with +32-shifted replicas).
  - o'[65, n] += v'^T_blk @ p_blk; row 64 (ones column) accumulates the
    softmax denominator. o' matmuls of iteration i run as PE "filler"
    inside iteration i+1's scores/exp phase; the v'^T projection itself
    runs as iteration 0's filler (no separate preamble phase).
  - softmax divide: both heads' denominators bounce through DRAM, one
    batched [2, n] reciprocal, per-head broadcast-DMA back, multiply.
  - out = wp_eff @ o; chunk 0's projection runs as iteration-5 filler.
"""

import math
import os
import sys

for _p in ("/opt/trn_rl_repo", "/root/.axon_site/_ro/trn_rl_repo"):
    if os.path.isdir(_p) and _p not in sys.path:
        sys.path.insert(0, _p)

import numpy as np

NUM_HEADS = 8
KEY_DIM = 16
D_HEAD = 64
B = 2
C = 256
HH = 56
WW = 56
N = HH * WW          # 3136 tokens
NCHUNK = N // 4      # 784 query tokens per core
NSUB = NCHUNK // 2   # 392, fits one PSUM bank
NB = (N + 127) // 128            # 25 key-blocks
MB_SIZES = [128] * 24 + [64]
KS = [128, 128]                  # contraction chunks for K=256
GROUPS = [list(range(g * 3, min(g * 3 + 3, NB))) for g in range(9)]
ACT_GROUPS = frozenset((0, 2, 4, 6, 8))  # exact-exp groups; rest on DVE

A16 = 128.0 / math.log(2.0)          # scale folded into wq: scores = A16*S
B_SCH = 16256.0 - 896.0 - 7.0        # Schraudolph bias incl. 2^-7 and c=7
ACT_SCALE = math.log(2.0) / 128.0    # undoes A16 on the exact-exp path
LN2_7 = -7.0 * math.log(2.0)         # exp bias; cancels in the divide

_GRAPH = None
DEBUG = False


def _build_graph():
    import concourse.bass as bass  # noqa: F401
    import concourse.mybir as mybir
    import concourse.tile as tile
    from concourse import bacc
    from contextlib import ExitStack

    f32 = mybir.dt.float32
    bf16 = mybir.dt.bfloat16
    i16 = mybir.dt.int16
    Exp = mybir.ActivationFunctionType.Exp

    nc = bacc.Bacc("TRN2", target_bir_lowering=False, debug=False, num_devices=8)
    xa_d = nc.dram_tensor("xa", [256, N], bf16, kind="ExternalInput").ap()
    xq_d = nc.dram_tensor("xq", [256, NCHUNK], bf16, kind="ExternalInput").ap()
    wq_d = nc.dram_tensor("wq", [256, 128], bf16, kind="ExternalInput").ap()
    wk_d = nc.dram_tensor("wk", [256, 128], bf16, kind="ExternalInput").ap()
    wv_d = nc.dram_tensor("wv", [256, 512], bf16, kind="ExternalInput").ap()
    qb_d = nc.dram_tensor("qb", [128, 1], f32, kind="ExternalInput").ap()
    pb_d = nc.dram_tensor("pb", [128, 2], f32, kind="ExternalInput").ap()
    wp_d = nc.dram_tensor("wp", [64, 8, 256], bf16, kind="ExternalInput").ap()
    out_d = nc.dram_tensor("out", [256, NCHUNK], f32, kind="ExternalOutput").ap()
    rsd_d = nc.dram_tensor("rsd", [16, NSUB], f32).ap()  # denominator bounce
    rsr_d = nc.dram_tensor("rsr", [16, NSUB], f32).ap()  # reciprocal bounce
    if DEBUG:
        dbg = {nm: nc.dram_tensor("dbg_" + nm, shp, dt, kind="ExternalOutput").ap()
               for nm, shp, dt in [
                   ("klo", [128, N], bf16), ("klo2", [128, N], bf16),
                   ("khi", [128, N], bf16), ("khi2", [128, N], bf16),
                   ("qlo", [128, NCHUNK], bf16), ("qlo2", [128, NCHUNK], bf16),
                   ("qhi", [128, NCHUNK], bf16), ("qhi2", [128, NCHUNK], bf16),
                   ("vt", [128, NB * 2 * 4 * 65], bf16),
                   ("of", [64, 8 * NCHUNK], bf16),
                   ("ksb", [128, N], bf16), ("qsb", [128, NCHUNK], bf16)]}

    with tile.TileContext(nc) as tc, ExitStack() as stk:
        const = stk.enter_context(tc.tile_pool(name="const", bufs=1))
        xq_sb = const.tile([128, 2, NCHUNK], bf16, tag="xq")
        wq_sb = const.tile([128, 2, 128], bf16, tag="wq")
        wk_sb = const.tile([128, 2, 128], bf16, tag="wk")
        wv_sb = const.tile([128, 2, 512], bf16, tag="wv")
        wp_sb = const.tile([64, 8, 256], bf16, tag="wp")
        qb_sb = const.tile([128, 1], f32, tag="qb")
        pb_sb = const.tile([128, 2], f32, tag="pb")
        eb_sb = const.tile([128, 1], f32, tag="eb")
        # per-head 32-aligned base partitions: head h -> (k_lo if h<4 else
        # k_hi) partitions [32*(h%4), 32*(h%4)+16)
        k_lo = const.tile([128, N], bf16, tag="klo")
        k_hi = const.tile([128, N], bf16, tag="khi")
        q_lo = const.tile([128, NCHUNK], bf16, tag="qlo")
        q_hi = const.tile([128, NCHUNK], bf16, tag="qhi")
        # replicas shifted by +32 partitions so consecutive blocks of one head
        # use different PE row groups (4-way concurrent scores)
        k_lo2 = const.tile([128, N], bf16, tag="klo2")
        k_hi2 = const.tile([128, N], bf16, tag="khi2")
        q_lo2 = const.tile([128, NCHUNK], bf16, tag="qlo2")
        q_hi2 = const.tile([128, NCHUNK], bf16, tag="qhi2")
        # v'^T: [m-in-block, block, half, head-in-half, 64 v cols + ones col]
        vT_sb = const.tile([128, NB, 2, 4, 65], bf16, tag="vt")
        of_sb = const.tile([64, 8, NCHUNK], bf16, tag="of")
        y_sb = const.tile([128, 2, NCHUNK], f32, tag="y")
        xa_sb = const.tile([128, 2, N], bf16, tag="xa")

        for kc in range(2):
            ks, off = KS[kc], 128 * kc
            nc.sync.dma_start(out=wq_sb[0:ks, kc, :], in_=wq_d[off:off + ks, :])
            nc.sync.dma_start(out=wk_sb[0:ks, kc, :], in_=wk_d[off:off + ks, :])
            nc.sync.dma_start(out=wv_sb[0:ks, kc, :], in_=wv_d[off:off + ks, :])
            nc.sync.dma_start(out=xq_sb[0:ks, kc, :], in_=xq_d[off:off + ks, :])
            nc.sync.dma_start(out=xa_sb[:, kc, :],
                              in_=xa_d[128 * kc:128 * kc + 128, :])
        nc.sync.dma_start(out=wp_sb[:], in_=wp_d[:])
        nc.sync.dma_start(out=qb_sb[:], in_=qb_d)
        nc.sync.dma_start(out=pb_sb[:], in_=pb_d)
        nc.vector.memset(eb_sb[:], LN2_7)
        nc.vector.memset(vT_sb[:, :, :, :, 64:65], 1.0)
        for t in (k_lo, k_hi, k_lo2, k_hi2, q_lo, q_hi, q_lo2, q_hi2):
            nc.vector.memset(t[:], 0.0)

        k_sb = const.tile([128, N], bf16, tag="ksb")
        q_sb = const.tile([128, NCHUNK], bf16, tag="qsb")

        with tc.tile_pool(name="pP", bufs=22) as pP, \
             tc.tile_pool(name="pEp", bufs=4) as pEp, \
             tc.tile_pool(name="psO", bufs=2, space="PSUM") as psO, \
             tc.tile_pool(name="psS", bufs=2, space="PSUM") as psS:

            # ---- projections: q then k (PE), evac on ACT / DVE ----
            for c2 in range(2):
                q_ps = psO.tile([128, 512], f32, tag="ops", name=f"qps{c2}")
                for kc in range(2):
                    nc.tensor.matmul(
                        q_ps[0:128, 0:NSUB],
                        wq_sb[0:KS[kc], kc, :],
                        xq_sb[0:KS[kc], kc, c2 * NSUB:(c2 + 1) * NSUB],
                        start=(kc == 0), stop=(kc == 1))
                nc.scalar.add(
                    q_sb[:, c2 * NSUB:(c2 + 1) * NSUB], q_ps[0:128, 0:NSUB],
                    qb_sb[:, 0:1])
            for h in range(8):
                qt = q_lo if h < 4 else q_hi
                qt2 = q_lo2 if h < 4 else q_hi2
                bp_ = 32 * (h % 4)
                bp2 = (bp_ + 32) % 128
                nc.gpsimd.dma_start(out=qt[bp_:bp_ + 16, :],
                                    in_=q_sb[16 * h:16 * h + 16, :])
                nc.gpsimd.dma_start(out=qt2[bp2:bp2 + 16, :],
                                    in_=q_sb[16 * h:16 * h + 16, :])
            for p in range(7):
                c0 = 512 * p
                cw = min(512, N - c0)
                k_ps = psO.tile([128, 512], f32, tag="ops", name=f"kps{p}")
                for kc in range(2):
                    nc.tensor.matmul(
                        k_ps[0:128, 0:cw],
                        wk_sb[0:KS[kc], kc, :],
                        xa_sb[0:KS[kc], kc, c0:c0 + cw],
                        start=(kc == 0), stop=(kc == 1))
                nc.vector.tensor_copy(k_sb[:, c0:c0 + cw], k_ps[0:128, 0:cw])
            for h in range(8):
                kt = k_lo if h < 4 else k_hi
                kt2 = k_lo2 if h < 4 else k_hi2
                bp_ = 32 * (h % 4)
                bp2 = (bp_ + 32) % 128
                eng = nc.sync if h % 2 == 0 else nc.gpsimd
                eng.dma_start(out=kt[bp_:bp_ + 16, :],
                              in_=k_sb[16 * h:16 * h + 16, :])
                eng.dma_start(out=kt2[bp2:bp2 + 16, :],
                              in_=k_sb[16 * h:16 * h + 16, :])

            # ---- main attention loop, software-pipelined ----
            # iteration i = (head-pair, n-chunk). During iteration i's
            # scores+exp phase the PE runs iteration i-1's o'-accumulation
            # matmuls as filler (iteration 0 runs the v'^T projection, and
            # iteration 5 additionally chunk 0's output projection).
            # PSUM: scores 2 x 3 banks + o'/v'/wp pool 2 x 1 bank = 8 banks.
            PAIRS = [(0, 2), (1, 3), (4, 6), (5, 7)]
            ITERS = [(pair, c2) for c2 in range(2) for pair in PAIRS]

            def emit_scores_group(c2, blocks, s_ps2, kts, qts, kts2, qts2,
                                  bps, bps2):
                nc0 = c2 * NSUB
                for i, mb in enumerate(blocks):
                    pbi = MB_SIZES[mb]
                    for e in range(2):
                        if mb % 2 == 0:
                            kte, qte, be = kts[e], qts[e], bps[e]
                        else:
                            kte, qte, be = kts2[e], qts2[e], bps2[e]
                        nc.tensor.matmul(
                            s_ps2[e][0:pbi, i, 0:NSUB],
                            kte[be:be + 32, mb * 128:mb * 128 + pbi],
                            qte[be:be + 32, nc0:nc0 + NSUB],
                            start=True, stop=True,
                            tile_position=(be, 0))

            def make_o_filler(pair, e, p_tile, i, mb, o_ps2):
                def emit():
                    h = pair[e]
                    pbi = MB_SIZES[mb]
                    nc.tensor.matmul(
                        o_ps2[e][0:65, 0:NSUB],
                        vT_sb[0:pbi, mb, h // 4, h % 4, :],
                        p_tile[0:pbi, i, 0:NSUB],
                        start=(mb == 0), stop=(mb == NB - 1))
                return emit

            def make_v_filler(mb):
                def emit():
                    pb_ = MB_SIZES[mb]
                    vt_ps = psO.tile([128, 2, 4, 64], f32, tag="ops",
                                     name=f"vtps{mb}")
                    # both halves share one PSUM bank: start=True clears
                    # has_written bank-wide, so only the first matmul may
                    # carry it (half 1's first write lands on cleared flags
                    # and overwrites; kc==1 then accumulates)
                    for kc in range(2):
                        for half in range(2):
                            nc.tensor.matmul(
                                vt_ps[0:pb_, half, :, :],
                                xa_sb[0:KS[kc], kc,
                                      mb * 128:mb * 128 + pb_],
                                wv_sb[0:KS[kc], kc,
                                      half * 256:(half + 1) * 256],
                                start=(kc == 0 and half == 0),
                                stop=(kc == 1), skip_group_check=True)
                    if mb % 2 == 0:
                        nc.vector.tensor_copy(
                            vT_sb[0:pb_, mb, :, :, 0:64], vt_ps[0:pb_, :, :, :])
                    else:
                        nc.scalar.copy(
                            vT_sb[0:pb_, mb, :, :, 0:64], vt_ps[0:pb_, :, :, :])
                return emit

            def make_wp_jobs(c2):
                # contract heads in pipeline-completion order: the last
                # pair's heads (5, 7) come last so earlier matmuls run while
                # the final epilogue's divide chain is still in flight
                nc0 = c2 * NSUB
                KC_ORDER = (0, 2, 1, 3, 4, 6, 5, 7)

                def job(mo):
                    def emit():
                        y_ps = psO.tile([128, 512], f32, tag="ops",
                                        name=f"yps{c2}{mo}")
                        for j, kc in enumerate(KC_ORDER):
                            nc.tensor.matmul(
                                y_ps[0:128, 0:NSUB],
                                wp_sb[0:64, kc, mo * 128:(mo + 1) * 128],
                                of_sb[0:64, kc, nc0:nc0 + NSUB],
                                start=(j == 0), stop=(j == 7))
                        nc.vector.tensor_scalar_add(
                            y_sb[:, mo, nc0:nc0 + NSUB], y_ps[0:128, 0:NSUB],
                            pb_sb[:, mo:mo + 1])
                        nc.sync.dma_start(
                            out=out_d[mo * 128:(mo + 1) * 128,
                                      nc0:nc0 + NSUB],
                            in_=y_sb[:, mo, nc0:nc0 + NSUB])
                    return emit
                return [job(0), job(1)]

            def emit_epilogue(pair, c2, o_ps2, it, last):
                nc0 = c2 * NSUB
                # evacuate PSUM promptly (rsh on ACT, ou on DVE) to free the
                # o' banks for the next iteration's fillers
                rsh = pEp.tile([128, 2, NSUB], f32, tag="rsh", bufs=2)
                ous = []
                for e in range(2):
                    nc.scalar.copy(rsh[64:65, e, :], o_ps2[e][64:65, 0:NSUB])
                    ou = pEp.tile([64, NSUB], f32, tag=f"ou{e}", name=f"ou{e}", bufs=2)
                    nc.vector.tensor_copy(ou[0:64, :], o_ps2[e][0:64, 0:NSUB])
                    ous.append(ou)
                nc.sync.dma_start(out=rsd_d[2 * it:2 * it + 2, :],
                                  in_=rsh[64:65, :, :])
                if not last:
                    # batched reciprocal of both heads' denominators
                    rb2 = pEp.tile([2, NSUB], f32, tag="rb2", bufs=2)
                    nc.sync.dma_start(out=rb2[0:2, :],
                                      in_=rsd_d[2 * it:2 * it + 2, :])
                    rbr2 = pEp.tile([2, NSUB], f32, tag="rbr2", bufs=2)
                    scr2 = pEp.tile([2, NSUB], f32, tag="scr2", bufs=2)
                    nc.vector.reciprocal_approx_accurate(
                        out=rbr2[0:2, :], in_=rb2[0:2, :], scratch=scr2[0:2, :])
                    nc.sync.dma_start(out=rsr_d[2 * it:2 * it + 2, :],
                                      in_=rbr2[0:2, :])
                    for e in range(2):
                        h = pair[e]
                        rbc = pEp.tile([64, NSUB], f32, tag="rbc")
                        nc.sync.dma_start(
                            out=rbc[0:64, :],
                            in_=rsr_d[2 * it + e:2 * it + e + 1,
                                      :].partition_broadcast(64))
                        nc.vector.tensor_mul(
                            out=of_sb[0:64, h, nc0:nc0 + NSUB],
                            in0=ous[e][0:64, :], in1=rbc[0:64, :])
                else:
                    # final iteration: broadcast den first (shorter latency
                    # chain), reciprocal per head
                    for e in range(2):
                        h = pair[e]
                        rbc = pEp.tile([64, NSUB], f32, tag="rbc")
                        nc.sync.dma_start(
                            out=rbc[0:64, :],
                            in_=rsd_d[2 * it + e:2 * it + e + 1,
                                      :].partition_broadcast(64))
                        rbr = pEp.tile([64, NSUB], f32, tag="rbrL", bufs=1)
                        scr = pEp.tile([64, NSUB], f32, tag="scrL", bufs=1)
                        nc.vector.reciprocal_approx_accurate(
                            out=rbr[0:64, :], in_=rbc[0:64, :],
                            scratch=scr[0:64, :])
                        nc.vector.tensor_mul(
                            out=of_sb[0:64, h, nc0:nc0 + NSUB],
                            in0=ous[e][0:64, :], in1=rbr[0:64, :])

            prev = None  # (pair, c2, p_tiles, it) of the previous iteration
            for it in range(len(ITERS) + 1):
                cur = ITERS[it] if it < len(ITERS) else None
                fillers = []
                if it == 0:
                    fillers = [make_v_filler(mb) for mb in range(NB)]
                if prev is not None:
                    ppair, pc2, p_tiles, pit = prev
                    o_ps2 = [psO.tile([128, 512], f32, tag="ops",
                                      name=f"ops{e}") for e in range(2)]
                    for g2, blocks2 in enumerate(GROUPS):
                        for i2, mb2 in enumerate(blocks2):
                            for e in range(2):
                                fillers.append(make_o_filler(
                                    ppair, e, p_tiles[g2][e], i2, mb2, o_ps2))
                if it == 5:
                    fillers.extend(make_wp_jobs(0))
                if cur is None:
                    for job in fillers:
                        job()
                    emit_epilogue(ppair, pc2, o_ps2, pit, last=True)
                    for job in make_wp_jobs(1):
                        job()
                    if DEBUG:
                        for nm, t in [("klo", k_lo), ("klo2", k_lo2),
                                      ("khi", k_hi), ("khi2", k_hi2),
                                      ("qlo", q_lo), ("qlo2", q_lo2),
                                      ("qhi", q_hi), ("qhi2", q_hi2),
                                      ("ksb", k_sb), ("qsb", q_sb)]:
                            nc.sync.dma_start(out=dbg[nm], in_=t[:])
                        nc.sync.dma_start(out=dbg["vt"], in_=vT_sb[:])
                        nc.sync.dma_start(out=dbg["of"], in_=of_sb[:])
                    break
                pair, c2 = cur
                kts = [k_lo if h < 4 else k_hi for h in pair]
                qts = [q_lo if h < 4 else q_hi for h in pair]
                kts2 = [k_lo2 if h < 4 else k_hi2 for h in pair]
                qts2 = [q_lo2 if h < 4 else q_hi2 for h in pair]
                bps = [32 * (h % 4) for h in pair]
                bps2 = [(32 * (h % 4) + 32) % 128 for h in pair]
                p_tiles = []
                nfill = len(fillers)
                for g, blocks in enumerate(GROUPS):
                    gsz = len(blocks)
                    pb_ = MB_SIZES[blocks[-1]]
                    s_ps2 = [psS.tile([128, 3, 512], f32, tag="sps",
                                      name=f"sps{e}") for e in range(2)]
                    emit_scores_group(c2, blocks, s_ps2, kts, qts,
                                      kts2, qts2, bps, bps2)
                    p_sb2 = [pP.tile([128, 3, NSUB], bf16, tag="psb",
                                     name=f"psb{e}") for e in range(2)]
                    for e in range(2):
                        if g in ACT_GROUPS:
                            nc.scalar.activation(
                                out=p_sb2[e][0:pb_, 0:gsz, 0:NSUB],
                                in_=s_ps2[e][0:pb_, 0:gsz, 0:NSUB], func=Exp,
                                bias=eb_sb[0:pb_, 0:1], scale=ACT_SCALE)
                        else:
                            nc.vector.tensor_scalar_add(
                                p_sb2[e][0:pb_, 0:gsz, 0:NSUB].bitcast(i16),
                                s_ps2[e][0:pb_, 0:gsz, 0:NSUB], B_SCH)
                    p_tiles.append(p_sb2)
                    # interleave previous iteration's o' matmuls as PE filler
                    ng = len(GROUPS) - 1
                    lo = nfill * max(0, g - 1) // ng
                    hi = nfill * g // ng
                    for job in fillers[lo:hi]:
                        job()
                if prev is not None:
                    emit_epilogue(ppair, pc2, o_ps2, pit, last=False)
                prev = (pair, c2, p_tiles, it)

    nc.compile()
    return nc


def get_graph():
    global _GRAPH
    if _GRAPH is None:
        _GRAPH = _build_graph()
    return _GRAPH


def make_in_maps(x, wq, sq, bq, wk, sk, bk, wv, sv, bv, wp, sp, bp):
    import ml_dtypes
    bf = ml_dtypes.bfloat16
    f = np.float32
    x2 = np.asarray(x, f).reshape(B, C, N)
    wq = np.asarray(wq, f); sq = np.asarray(sq, f); bq = np.asarray(bq, f)
    wk = np.asarray(wk, f); sk = np.asarray(sk, f)
    wv = np.asarray(wv, f); sv = np.asarray(sv, f); bv = np.asarray(bv, f)
    wp = np.asarray(wp, f); sp = np.asarray(sp, f); bp = np.asarray(bp, f)

    wq_eff = ((wq * sq[:, None]).T * A16).astype(f)   # (256, 128), pre-scaled
    wk_eff = (wk * sk[:, None]).T.astype(f)           # k-bias dropped
    wv_base = wv * sv[:, None]  # (512, 256)
    wv_arr = np.zeros((256, 512), f)
    for h in range(NUM_HEADS):
        col = 256 * (h // 4) + 64 * (h % 4)
        wv_arr[:, col:col + 64] = wv_base[64 * h:64 * h + 64, :].T
    wp_sc = wp * sp[:, None]                 # (256, 512)
    wp_eff = wp_sc.T.astype(f)               # (512, 256), row c = 64h+d
    wp_arr = wp_eff.reshape(8, 64, 256).transpose(1, 0, 2).copy()
    pb_fold = (bp + wp_sc @ bv).astype(f)    # v-bias folded into out-bias
    pb_arr = pb_fold.reshape(2, 128).T.copy()  # pb_arr[d, mo] = pb'[128*mo+d]
    in_maps = []
    for core in range(8):
        b, j = core // 4, core % 4
        xa_full = np.ascontiguousarray(x2[b])
        xq_c = np.ascontiguousarray(xa_full[:, j * NCHUNK:(j + 1) * NCHUNK])
        in_maps.append(dict(
            xa=xa_full.astype(bf), xq=xq_c.astype(bf),
            wq=wq_eff.astype(bf), wk=wk_eff.astype(bf),
            wv=wv_arr.astype(bf), wp=wp_arr.astype(bf),
            qb=(bq * A16).reshape(128, 1).astype(f),
            pb=pb_arr.astype(f)))
    return in_maps


def assemble_output(results):
    y = np.zeros((B, C, N), np.float32)
    for core in range(8):
        b, j = core // 4, core % 4
        y[b, :, j * NCHUNK:(j + 1) * NCHUNK] = results[core]["out"]
    return y.reshape(B, C, HH, WW)


def kernel(**inputs):
    from concourse.bass_utils import run_bass_kernel_spmd
    nc = get_graph()
    in_maps = make_in_maps(**inputs)
    res = run_bass_kernel_spmd(nc, in_maps, core_ids=list(range(8)))
    return assemble_output(res.results)


if __name__ == "__main__":
    rng = np.random.default_rng(0)
    ins = dict(
        x=rng.standard_normal((2, 256, 56, 56), np.float32),
        wq=rng.standard_normal((128, 256), np.float32) * 0.05,
        sq=rng.random(128, np.float32),
        bq=rng.standard_normal(128, np.float32) * 0.05,
        wk=rng.standard_normal((128, 256), np.float32) * 0.05,
        sk=rng.random(128, np.float32),
        bk=rng.standard_normal(128, np.float32) * 0.05,
        wv=rng.standard_normal((512, 256), np.float32) * 0.05,
        sv=rng.random(512, np.float32),
        bv=rng.standard_normal(512, np.float32) * 0.05,
        wp=rng.standard_normal((256, 512), np.float32) * 0.05,
        sp=rng.random(256, np.float32),
        bp=rng.standard_normal(256, np.float32) * 0.05,
    )
    out = kernel(**ins)
    print("out", out.shape, out.dtype, float(np.abs(out).mean()))


# revision 8
# speedup vs baseline: 1.1650x; 1.0265x over previous
"""Trainium2 Bass kernel for nn_Attention (dense transformer attention block).

Reference computation (per batch b):
  q = BN(wq @ x) -> (8 heads, 16, 3136);  k likewise;  v = BN(wv @ x) -> (8, 64, 3136)
  attn = softmax(q^T k) over 3136x3136 tokens (no 1/sqrt(d) scaling)
  o = attn @ v^T -> (512, 56, 56);  out = BN(wp @ o) -> (256, 56, 56)

Sharding: 8 cores = 2 batches x 4 query-token chunks of 784. Each core
computes k/v for all 3136 key tokens (cheap, redundant) and attention +
output projection for its own 784 query tokens. Zero collectives; the host
assembles the 8 output shards.

Device algorithm per core (flash-style; bf16 matmuls, f32 PSUM accumulation):
  - The softmax exp (the serial bottleneck: 19.7M elements/core, ACT-only
    at 1 elem/cycle/lane) is split across TWO engines: even score-groups run
    exact exp on ACT; odd groups run a one-op Schraudolph bit-trick on DVE
    (int16(a*S + b) bit-cast to bf16 ~= exp(S)*2^-7). The a=128/ln2 scale is
    folded into wq/bq host-side; ACT's activation-scale undoes it for the
    exact path. The 2^-7 factor cancels in the softmax divide.
  - k-bias is dropped entirely (a per-query-row score shift, softmax-
    invariant); v-bias is folded into the output projection bias host-side
    (out += wp_eff @ bv), so v' evacuation is a plain copy.
  - Scores S_T[m, n] = k_blk^T q (K=16) run as 4-way concurrent PE
    row-group tiles (tile_position packing with +32-shifted replicas).
  - o'[65, n] += v'^T_blk @ p_blk; row 64 (ones column) accumulates the
    softmax denominator. o' matmuls of iteration i run as PE "filler"
    inside iteration i+1's scores/exp phase; the v'^T projection itself
    runs as iteration 0's filler (no separate preamble phase).
  - softmax divide: both heads' denominators bounce through DRAM, one
    batched [2, n] reciprocal, per-head broadcast-DMA back, multiply.
  - out = wp_eff @ o; chunk 0's projection runs as iteration-5 filler.
"""

import math
import os
import sys

for _p in ("/opt/trn_rl_repo", "/root/.axon_site/_ro/trn_rl_repo"):
    if os.path.isdir(_p) and _p not in sys.path:
        sys.path.insert(0, _p)

import numpy as np

NUM_HEADS = 8
KEY_DIM = 16
D_HEAD = 64
B = 2
C = 256
HH = 56
WW = 56
N = HH * WW          # 3136 tokens
NCHUNK = N // 4      # 784 query tokens per core
NSUB = NCHUNK // 2   # 392, fits one PSUM bank
NB = (N + 127) // 128            # 25 key-blocks
MB_SIZES = [128] * 24 + [64]
KS = [128, 128]                  # contraction chunks for K=256
GROUPS = [list(range(g * 3, min(g * 3 + 3, NB))) for g in range(9)]
ACT_GROUPS = frozenset((0, 2, 4, 6, 8))  # exact-exp groups; rest on DVE

A16 = 128.0 / math.log(2.0)          # scale folded into wq: scores = A16*S
B_SCH = 16256.0 - 896.0 - 7.0        # Schraudolph bias incl. 2^-7 and c=7
ACT_SCALE = math.log(2.0) / 128.0    # undoes A16 on the exact-exp path
LN2_7 = -7.0 * math.log(2.0)         # exp bias; cancels in the divide

_GRAPH = None
DEBUG = False


def _build_graph():
    import concourse.bass as bass  # noqa: F401
    import concourse.mybir as mybir
    import concourse.tile as tile
    from concourse import bacc
    from contextlib import ExitStack

    f32 = mybir.dt.float32
    bf16 = mybir.dt.bfloat16
    i16 = mybir.dt.int16
    Exp = mybir.ActivationFunctionType.Exp

    nc = bacc.Bacc("TRN2", target_bir_lowering=False, debug=False, num_devices=8)
    xa_d = nc.dram_tensor("xa", [256, N], bf16, kind="ExternalInput").ap()
    xq_d = nc.dram_tensor("xq", [256, NCHUNK], bf16, kind="ExternalInput").ap()
    wq_d = nc.dram_tensor("wq", [256, 128], bf16, kind="ExternalInput").ap()
    wk_d = nc.dram_tensor("wk", [256, 128], bf16, kind="ExternalInput").ap()
    wv_d = nc.dram_tensor("wv", [256, 512], bf16, kind="ExternalInput").ap()
    qb_d = nc.dram_tensor("qb", [128, 1], f32, kind="ExternalInput").ap()
    pb_d = nc.dram_tensor("pb", [128, 2], f32, kind="ExternalInput").ap()
    wp_d = nc.dram_tensor("wp", [64, 8, 256], bf16, kind="ExternalInput").ap()
    out_d = nc.dram_tensor("out", [256, NCHUNK], f32, kind="ExternalOutput").ap()
    rsd_d = nc.dram_tensor("rsd", [16, NSUB], f32).ap()  # denominator bounce
    rsr_d = nc.dram_tensor("rsr", [16, NSUB], f32).ap()  # reciprocal bounce
    if DEBUG:
        dbg = {nm: nc.dram_tensor("dbg_" + nm, shp, dt, kind="ExternalOutput").ap()
               for nm, shp, dt in [
                   ("klo", [128, N], bf16), ("klo2", [128, N], bf16),
                   ("khi", [128, N], bf16), ("khi2", [128, N], bf16),
                   ("qlo", [128, NCHUNK], bf16), ("qlo2", [128, NCHUNK], bf16),
                   ("qhi", [128, NCHUNK], bf16), ("qhi2", [128, NCHUNK], bf16),
                   ("vt", [128, NB * 2 * 4 * 65], bf16),
                   ("of", [64, 8 * NCHUNK], bf16),
                   ("ksb", [128, N], bf16), ("qsb", [128, NCHUNK], bf16)]}

    with tile.TileContext(nc) as tc, ExitStack() as stk:
        const = stk.enter_context(tc.tile_pool(name="const", bufs=1))
        xq_sb = const.tile([128, 2, NCHUNK], bf16, tag="xq")
        wq_sb = const.tile([128, 2, 128], bf16, tag="wq")
        wk_sb = const.tile([128, 2, 128], bf16, tag="wk")
        wv_sb = const.tile([128, 2, 512], bf16, tag="wv")
        wp_sb = const.tile([64, 8, 256], bf16, tag="wp")
        qb_sb = const.tile([128, 1], f32, tag="qb")
        pb_sb = const.tile([128, 2], f32, tag="pb")
        eb_sb = const.tile([128, 1], f32, tag="eb")
        # per-head 32-aligned base partitions: head h -> (k_lo if h<4 else
        # k_hi) partitions [32*(h%4), 32*(h%4)+16)
        k_lo = const.tile([128, N], bf16, tag="klo")
        k_hi = const.tile([128, N], bf16, tag="khi")
        q_lo = const.tile([128, NCHUNK], bf16, tag="qlo")
        q_hi = const.tile([128, NCHUNK], bf16, tag="qhi")
        # replicas shifted by +32 partitions so consecutive blocks of one head
        # use different PE row groups (4-way concurrent scores)
        k_lo2 = const.tile([128, N], bf16, tag="klo2")
        k_hi2 = const.tile([128, N], bf16, tag="khi2")
        q_lo2 = const.tile([128, NCHUNK], bf16, tag="qlo2")
        q_hi2 = const.tile([128, NCHUNK], bf16, tag="qhi2")
        # v'^T: [m-in-block, block, half, head-in-half, 64 v cols + ones col]
        vT_sb = const.tile([128, NB, 2, 4, 65], bf16, tag="vt")
        of_sb = const.tile([64, 8, NCHUNK], bf16, tag="of")
        y_sb = const.tile([128, 2, NCHUNK], f32, tag="y")
        xa_sb = const.tile([128, 2, N], bf16, tag="xa")

        for kc in range(2):
            ks, off = KS[kc], 128 * kc
            nc.sync.dma_start(out=wq_sb[0:ks, kc, :], in_=wq_d[off:off + ks, :])
            nc.sync.dma_start(out=wk_sb[0:ks, kc, :], in_=wk_d[off:off + ks, :])
            nc.sync.dma_start(out=wv_sb[0:ks, kc, :], in_=wv_d[off:off + ks, :])
            nc.sync.dma_start(out=xq_sb[0:ks, kc, :], in_=xq_d[off:off + ks, :])
            nc.gpsimd.dma_start(out=xa_sb[:, kc, :],
                              in_=xa_d[128 * kc:128 * kc + 128, :])
        nc.sync.dma_start(out=wp_sb[:], in_=wp_d[:])
        nc.sync.dma_start(out=qb_sb[:], in_=qb_d)
        nc.sync.dma_start(out=pb_sb[:], in_=pb_d)
        nc.vector.memset(eb_sb[:], LN2_7)
        nc.vector.memset(vT_sb[:, :, :, :, 64:65], 1.0)
        for t in (k_lo, k_hi, k_lo2, k_hi2):
            nc.gpsimd.memset(t[:], 0.0)
        for t in (q_lo, q_hi, q_lo2, q_hi2):
            nc.vector.memset(t[:], 0.0)

        k_sb = const.tile([128, N], bf16, tag="ksb")
        q_sb = const.tile([128, NCHUNK], bf16, tag="qsb")

        with tc.tile_pool(name="pP", bufs=22) as pP, \
             tc.tile_pool(name="pEp", bufs=4) as pEp, \
             tc.tile_pool(name="psO", bufs=2, space="PSUM") as psO, \
             tc.tile_pool(name="psS", bufs=2, space="PSUM") as psS:

            # ---- projections: q then k (PE), evac on ACT / DVE ----
            for c2 in range(2):
                q_ps = psO.tile([128, 512], f32, tag="ops", name=f"qps{c2}")
                for kc in range(2):
                    nc.tensor.matmul(
                        q_ps[0:128, 0:NSUB],
                        wq_sb[0:KS[kc], kc, :],
                        xq_sb[0:KS[kc], kc, c2 * NSUB:(c2 + 1) * NSUB],
                        start=(kc == 0), stop=(kc == 1))
                nc.scalar.add(
                    q_sb[:, c2 * NSUB:(c2 + 1) * NSUB], q_ps[0:128, 0:NSUB],
                    qb_sb[:, 0:1])
            for h in range(8):
                qt = q_lo if h < 4 else q_hi
                qt2 = q_lo2 if h < 4 else q_hi2
                bp_ = 32 * (h % 4)
                bp2 = (bp_ + 32) % 128
                nc.gpsimd.dma_start(out=qt[bp_:bp_ + 16, :],
                                    in_=q_sb[16 * h:16 * h + 16, :])
                nc.gpsimd.dma_start(out=qt2[bp2:bp2 + 16, :],
                                    in_=q_sb[16 * h:16 * h + 16, :])
            for p in range(7):
                c0 = 512 * p
                cw = min(512, N - c0)
                k_ps = psO.tile([128, 512], f32, tag="ops", name=f"kps{p}")
                for kc in range(2):
                    nc.tensor.matmul(
                        k_ps[0:128, 0:cw],
                        wk_sb[0:KS[kc], kc, :],
                        xa_sb[0:KS[kc], kc, c0:c0 + cw],
                        start=(kc == 0), stop=(kc == 1))
                nc.vector.tensor_copy(k_sb[:, c0:c0 + cw], k_ps[0:128, 0:cw])
            for h in range(8):
                kt = k_lo if h < 4 else k_hi
                kt2 = k_lo2 if h < 4 else k_hi2
                bp_ = 32 * (h % 4)
                bp2 = (bp_ + 32) % 128
                eng = nc.sync if h % 2 == 0 else nc.gpsimd
                eng.dma_start(out=kt[bp_:bp_ + 16, :],
                              in_=k_sb[16 * h:16 * h + 16, :])
                eng.dma_start(out=kt2[bp2:bp2 + 16, :],
                              in_=k_sb[16 * h:16 * h + 16, :])

            # ---- main attention loop, software-pipelined ----
            # iteration i = (head-pair, n-chunk). During iteration i's
            # scores+exp phase the PE runs iteration i-1's o'-accumulation
            # matmuls as filler (iteration 0 runs the v'^T projection, and
            # iteration 5 additionally chunk 0's output projection).
            # PSUM: scores 2 x 3 banks + o'/v'/wp pool 2 x 1 bank = 8 banks.
            PAIRS = [(0, 2), (1, 3), (4, 6), (5, 7)]
            ITERS = [(pair, c2) for c2 in range(2) for pair in PAIRS]

            def emit_scores_group(c2, blocks, s_ps2, kts, qts, kts2, qts2,
                                  bps, bps2):
                nc0 = c2 * NSUB
                for i, mb in enumerate(blocks):
                    pbi = MB_SIZES[mb]
                    for e in range(2):
                        if mb % 2 == 0:
                            kte, qte, be = kts[e], qts[e], bps[e]
                        else:
                            kte, qte, be = kts2[e], qts2[e], bps2[e]
                        nc.tensor.matmul(
                            s_ps2[e][0:pbi, i, 0:NSUB],
                            kte[be:be + 32, mb * 128:mb * 128 + pbi],
                            qte[be:be + 32, nc0:nc0 + NSUB],
                            start=True, stop=True,
                            tile_position=(be, 0))

            def make_o_filler(pair, e, p_tile, i, mb, o_ps2):
                def emit():
                    h = pair[e]
                    pbi = MB_SIZES[mb]
                    nc.tensor.matmul(
                        o_ps2[e][0:65, 0:NSUB],
                        vT_sb[0:pbi, mb, h // 4, h % 4, :],
                        p_tile[0:pbi, i, 0:NSUB],
                        start=(mb == 0), stop=(mb == NB - 1))
                return emit

            def make_v_filler(mb):
                def emit():
                    pb_ = MB_SIZES[mb]
                    vt_ps = psO.tile([128, 2, 4, 64], f32, tag="ops",
                                     name=f"vtps{mb}")
                    # both halves share one PSUM bank: start=True clears
                    # has_written bank-wide, so only the first matmul may
                    # carry it (half 1's first write lands on cleared flags
                    # and overwrites; kc==1 then accumulates)
                    for kc in range(2):
                        for half in range(2):
                            nc.tensor.matmul(
                                vt_ps[0:pb_, half, :, :],
                                xa_sb[0:KS[kc], kc,
                                      mb * 128:mb * 128 + pb_],
                                wv_sb[0:KS[kc], kc,
                                      half * 256:(half + 1) * 256],
                                start=(kc == 0 and half == 0),
                                stop=(kc == 1), skip_group_check=True)
                    if mb % 2 == 0:
                        nc.vector.tensor_copy(
                            vT_sb[0:pb_, mb, :, :, 0:64], vt_ps[0:pb_, :, :, :])
                    else:
                        nc.scalar.copy(
                            vT_sb[0:pb_, mb, :, :, 0:64], vt_ps[0:pb_, :, :, :])
                return emit

            def make_wp_jobs(c2):
                # contract heads in pipeline-completion order: the last
                # pair's heads (5, 7) come last so earlier matmuls run while
                # the final epilogue's divide chain is still in flight
                nc0 = c2 * NSUB
                KC_ORDER = (0, 2, 1, 3, 4, 6, 5, 7)

                def job(mo):
                    def emit():
                        y_ps = psO.tile([128, 512], f32, tag="ops",
                                        name=f"yps{c2}{mo}")
                        for j, kc in enumerate(KC_ORDER):
                            nc.tensor.matmul(
                                y_ps[0:128, 0:NSUB],
                                wp_sb[0:64, kc, mo * 128:(mo + 1) * 128],
                                of_sb[0:64, kc, nc0:nc0 + NSUB],
                                start=(j == 0), stop=(j == 7))
                        nc.vector.tensor_scalar_add(
                            y_sb[:, mo, nc0:nc0 + NSUB], y_ps[0:128, 0:NSUB],
                            pb_sb[:, mo:mo + 1])
                        nc.sync.dma_start(
                            out=out_d[mo * 128:(mo + 1) * 128,
                                      nc0:nc0 + NSUB],
                            in_=y_sb[:, mo, nc0:nc0 + NSUB])
                    return emit
                return [job(0), job(1)]

            def emit_epilogue(pair, c2, o_ps2, it, last):
                nc0 = c2 * NSUB
                # evacuate PSUM promptly (rsh on ACT, ou on DVE) to free the
                # o' banks for the next iteration's fillers
                rsh = pEp.tile([128, 2, NSUB], f32, tag="rsh", bufs=2)
                ous = []
                for e in range(2):
                    nc.scalar.copy(rsh[64:65, e, :], o_ps2[e][64:65, 0:NSUB])
                    ou = pEp.tile([64, NSUB], f32, tag=f"ou{e}", name=f"ou{e}", bufs=2)
                    if e == 0:
                        nc.scalar.copy(ou[0:64, :], o_ps2[e][0:64, 0:NSUB])
                    else:
                        nc.vector.tensor_copy(ou[0:64, :], o_ps2[e][0:64, 0:NSUB])
                    ous.append(ou)
                nc.sync.dma_start(out=rsd_d[2 * it:2 * it + 2, :],
                                  in_=rsh[64:65, :, :])
                if not last:
                    # batched reciprocal of both heads' denominators
                    rb2 = pEp.tile([2, NSUB], f32, tag="rb2", bufs=2)
                    nc.sync.dma_start(out=rb2[0:2, :],
                                      in_=rsd_d[2 * it:2 * it + 2, :])
                    rbr2 = pEp.tile([2, NSUB], f32, tag="rbr2", bufs=2)
                    scr2 = pEp.tile([2, NSUB], f32, tag="scr2", bufs=2)
                    nc.vector.reciprocal_approx_accurate(
                        out=rbr2[0:2, :], in_=rb2[0:2, :], scratch=scr2[0:2, :])
                    nc.sync.dma_start(out=rsr_d[2 * it:2 * it + 2, :],
                                      in_=rbr2[0:2, :])
                    for e in range(2):
                        h = pair[e]
                        rbc = pEp.tile([64, NSUB], f32, tag="rbc")
                        nc.sync.dma_start(
                            out=rbc[0:64, :],
                            in_=rsr_d[2 * it + e:2 * it + e + 1,
                                      :].partition_broadcast(64))
                        nc.vector.tensor_mul(
                            out=of_sb[0:64, h, nc0:nc0 + NSUB],
                            in0=ous[e][0:64, :], in1=rbc[0:64, :])
                else:
                    # final iteration: broadcast den first (shorter latency
                    # chain), reciprocal per head
                    for e in range(2):
                        h = pair[e]
                        rbc = pEp.tile([64, NSUB], f32, tag="rbc")
                        nc.sync.dma_start(
                            out=rbc[0:64, :],
                            in_=rsd_d[2 * it + e:2 * it + e + 1,
                                      :].partition_broadcast(64))
                        rbr = pEp.tile([64, NSUB], f32, tag="rbrL", bufs=1)
                        scr = pEp.tile([64, NSUB], f32, tag="scrL", bufs=1)
                        nc.vector.reciprocal_approx_accurate(
                            out=rbr[0:64, :], in_=rbc[0:64, :],
                            scratch=scr[0:64, :])
                        nc.vector.tensor_mul(
                            out=of_sb[0:64, h, nc0:nc0 + NSUB],
                            in0=ous[e][0:64, :], in1=rbr[0:64, :])

            prev = None  # (pair, c2, p_tiles, it) of the previous iteration
            for it in range(len(ITERS) + 1):
                cur = ITERS[it] if it < len(ITERS) else None
                fillers = []
                if it == 0:
                    fillers = [make_v_filler(mb) for mb in range(15)]
                if it == 1:
                    fillers = [make_v_filler(mb) for mb in range(15, NB)]
                if prev is not None:
                    ppair, pc2, p_tiles, pit = prev
                    o_ps2 = [psO.tile([128, 512], f32, tag="ops",
                                      name=f"ops{e}") for e in range(2)]
                    for g2, blocks2 in enumerate(GROUPS):
                        for i2, mb2 in enumerate(blocks2):
                            for e in range(2):
                                fillers.append(make_o_filler(
                                    ppair, e, p_tiles[g2][e], i2, mb2, o_ps2))
                if it == 5:
                    fillers.extend(make_wp_jobs(0))
                if cur is None:
                    for job in fillers:
                        job()
                    emit_epilogue(ppair, pc2, o_ps2, pit, last=True)
                    for job in make_wp_jobs(1):
                        job()
                    if DEBUG:
                        for nm, t in [("klo", k_lo), ("klo2", k_lo2),
                                      ("khi", k_hi), ("khi2", k_hi2),
                                      ("qlo", q_lo), ("qlo2", q_lo2),
                                      ("qhi", q_hi), ("qhi2", q_hi2),
                                      ("ksb", k_sb), ("qsb", q_sb)]:
                            nc.sync.dma_start(out=dbg[nm], in_=t[:])
                        nc.sync.dma_start(out=dbg["vt"], in_=vT_sb[:])
                        nc.sync.dma_start(out=dbg["of"], in_=of_sb[:])
                    break
                pair, c2 = cur
                kts = [k_lo if h < 4 else k_hi for h in pair]
                qts = [q_lo if h < 4 else q_hi for h in pair]
                kts2 = [k_lo2 if h < 4 else k_hi2 for h in pair]
                qts2 = [q_lo2 if h < 4 else q_hi2 for h in pair]
                bps = [32 * (h % 4) for h in pair]
                bps2 = [(32 * (h % 4) + 32) % 128 for h in pair]
                p_tiles = []
                nfill = len(fillers)
                for g, blocks in enumerate(GROUPS):
                    gsz = len(blocks)
                    pb_ = MB_SIZES[blocks[-1]]
                    s_ps2 = [psS.tile([128, 3, 512], f32, tag="sps",
                                      name=f"sps{e}") for e in range(2)]
                    emit_scores_group(c2, blocks, s_ps2, kts, qts,
                                      kts2, qts2, bps, bps2)
                    p_sb2 = [pP.tile([128, 3, NSUB], bf16, tag="psb",
                                     name=f"psb{e}") for e in range(2)]
                    for e in range(2):
                        if g in ACT_GROUPS:
                            nc.scalar.activation(
                                out=p_sb2[e][0:pb_, 0:gsz, 0:NSUB],
                                in_=s_ps2[e][0:pb_, 0:gsz, 0:NSUB], func=Exp,
                                bias=eb_sb[0:pb_, 0:1], scale=ACT_SCALE)
                        else:
                            nc.vector.tensor_scalar_add(
                                p_sb2[e][0:pb_, 0:gsz, 0:NSUB].bitcast(i16),
                                s_ps2[e][0:pb_, 0:gsz, 0:NSUB], B_SCH)
                    p_tiles.append(p_sb2)
                    # interleave previous iteration's o' matmuls as PE
                    # filler; finish by slot 7 so the epilogue (which frees
                    # the o' PSUM banks) runs before the iteration ends
                    ng = len(GROUPS) - 2
                    lo = nfill * max(0, g - 1) // ng
                    hi = nfill * min(g, ng) // ng
                    for job in fillers[lo:hi]:
                        job()
                    if g == 7 and prev is not None:
                        emit_epilogue(ppair, pc2, o_ps2, pit, last=False)
                prev = (pair, c2, p_tiles, it)

    nc.compile()
    return nc


def get_graph():
    global _GRAPH
    if _GRAPH is None:
        _GRAPH = _build_graph()
    return _GRAPH


def make_in_maps(x, wq, sq, bq, wk, sk, bk, wv, sv, bv, wp, sp, bp):
    import ml_dtypes
    bf = ml_dtypes.bfloat16
    f = np.float32
    x2 = np.asarray(x, f).reshape(B, C, N)
    wq = np.asarray(wq, f); sq = np.asarray(sq, f); bq = np.asarray(bq, f)
    wk = np.asarray(wk, f); sk = np.asarray(sk, f)
    wv = np.asarray(wv, f); sv = np.asarray(sv, f); bv = np.asarray(bv, f)
    wp = np.asarray(wp, f); sp = np.asarray(sp, f); bp = np.asarray(bp, f)

    wq_eff = ((wq * sq[:, None]).T * A16).astype(f)   # (256, 128), pre-scaled
    wk_eff = (wk * sk[:, None]).T.astype(f)           # k-bias dropped
    wv_base = wv * sv[:, None]  # (512, 256)
    wv_arr = np.zeros((256, 512), f)
    for h in range(NUM_HEADS):
        col = 256 * (h // 4) + 64 * (h % 4)
        wv_arr[:, col:col + 64] = wv_base[64 * h:64 * h + 64, :].T
    wp_sc = wp * sp[:, None]                 # (256, 512)
    wp_eff = wp_sc.T.astype(f)               # (512, 256), row c = 64h+d
    wp_arr = wp_eff.reshape(8, 64, 256).transpose(1, 0, 2).copy()
    pb_fold = (bp + wp_sc @ bv).astype(f)    # v-bias folded into out-bias
    pb_arr = pb_fold.reshape(2, 128).T.copy()  # pb_arr[d, mo] = pb'[128*mo+d]
    in_maps = []
    for core in range(8):
        b, j = core // 4, core % 4
        xa_full = np.ascontiguousarray(x2[b])
        xq_c = np.ascontiguousarray(xa_full[:, j * NCHUNK:(j + 1) * NCHUNK])
        in_maps.append(dict(
            xa=xa_full.astype(bf), xq=xq_c.astype(bf),
            wq=wq_eff.astype(bf), wk=wk_eff.astype(bf),
            wv=wv_arr.astype(bf), wp=wp_arr.astype(bf),
            qb=(bq * A16).reshape(128, 1).astype(f),
            pb=pb_arr.astype(f)))
    return in_maps


def assemble_output(results):
    y = np.zeros((B, C, N), np.float32)
    for core in range(8):
        b, j = core // 4, core % 4
        y[b, :, j * NCHUNK:(j + 1) * NCHUNK] = results[core]["out"]
    return y.reshape(B, C, HH, WW)


def kernel(**inputs):
    from concourse.bass_utils import run_bass_kernel_spmd
    nc = get_graph()
    in_maps = make_in_maps(**inputs)
    res = run_bass_kernel_spmd(nc, in_maps, core_ids=list(range(8)))
    return assemble_output(res.results)


if __name__ == "__main__":
    rng = np.random.default_rng(0)
    ins = dict(
        x=rng.standard_normal((2, 256, 56, 56), np.float32),
        wq=rng.standard_normal((128, 256), np.float32) * 0.05,
        sq=rng.random(128, np.float32),
        bq=rng.standard_normal(128, np.float32) * 0.05,
        wk=rng.standard_normal((128, 256), np.float32) * 0.05,
        sk=rng.random(128, np.float32),
        bk=rng.standard_normal(128, np.float32) * 0.05,
        wv=rng.standard_normal((512, 256), np.float32) * 0.05,
        sv=rng.random(512, np.float32),
        bv=rng.standard_normal(512, np.float32) * 0.05,
        wp=rng.standard_normal((256, 512), np.float32) * 0.05,
        sp=rng.random(256, np.float32),
        bp=rng.standard_normal(256, np.float32) * 0.05,
    )
    out = kernel(**ins)
    print("out", out.shape, out.dtype, float(np.abs(out).mean()))
